# revision 24
# baseline (speedup 1.0000x reference)
"""Trainium2 Bass kernel for nn_GCNWithMultiHeadGATAndTCN_42356967473538.

Sharding: 8 cores = (batch b in 0..3) x (node-half s in 0..1).
Each core computes its 1024 node rows of its batch through the whole
pipeline, channels-major ([channel partitions, node free]) so BatchNorm
scales are per-partition and the TCN conv contracts on partitions.

Cross-core communication (training-mode BatchNorm couples all batches):
  C1: AllReduce [128,4]   bn1 sums           (all 8 cores)
  C2: AllGather [128,2048] h_bn^T            (pairs: other node half)
  C3: AllGather [128,4]   g boundary columns (pairs: conv halo)
  C4: AllReduce [128,4]   bn2 sums           (all 8 cores)
"""

import numpy as np

import concourse.bass as bass
import concourse.mybir as mybir
import concourse.tile as tile
from concourse import bacc
from concourse.bass_utils import run_bass_kernel_spmd

F32 = mybir.dt.float32
F32R = mybir.dt.float32r
AF = mybir.ActivationFunctionType
ALU = mybir.AluOpType
AX = mybir.AxisListType

B, N, FEAT, C, H, DH = 4, 2048, 256, 256, 4, 64
P = 128
R = N // 2            # own rows per core (1024)
NC = 8                # cores
EPS = 1e-5
SLOPE = 0.2
EXP_SHIFT = 64.0  # softmax-invariant constant shift: keeps exp in f32 range
CNT = float(B * N)    # batchnorm sample count (8192)

PAIRS = [[0, 1], [2, 3], [4, 5], [6, 7]]
ALL8 = [list(range(NC))]


def _bc_ap(ap, parts=P):
    """Broadcast a DRAM AP across `parts` partitions (stride-0 partition dim)."""
    return bass.AP(tensor=ap.tensor, offset=ap.offset, ap=[[0, parts], *ap.ap])


def build_program(alpha_gat: float, alpha_tcn: float, sim_safe: bool = False,
                  debug_taps: bool = False):
    nc = bacc.Bacc(
        "TRN2", target_bir_lowering=False, debug=False, num_devices=NC
    )

    def din(name, shape, dt=F32):
        return nc.dram_tensor(name, shape, dt, kind="ExternalInput").ap()

    xT = din("xT", [FEAT, N], F32R)      # x[b].T
    adjTc = din("adjTc", [N, R], F32R)   # adj[s*R:(s+1)*R, :].T  (own columns)
    W = din("W", [FEAT, C], F32R)        # W_sage
    bs = din("bs", [C])
    g1 = din("g1", [C])
    b1 = din("b1", [C])
    Whp = din("Whp", [C, H * DH], F32R)        # Wh packed [j, h*64+d]
    WkT = din("WkT", [3, C, C], F32R)          # conv_w[:, :, k].T -> [k, cin, cout]
    g2 = din("g2", [C])
    b2 = din("b2", [C])
    Ebc = din("Ebc", [4, C], F32R)             # head->channel one-hot (recip bcast)
    hmask = din("hmask", [2, 2, 2])      # halo select [L/R, src, first/last]

    out = nc.dram_tensor("out", [P, 2, R], F32, kind="ExternalOutput").ap()
    taps = {}
    if debug_taps:
        for tn, shape in [("d_hT", [P, 2, R]), ("d_hbnT", [P, 2, R]),
                          ("d_hpTo", [P, 2, R]), ("d_aggT", [P, 2, R]),
                          ("d_den4", [4, 2 * 512]), ("d_g", [P, 2, R + 2]),
                          ("d_tconv", [P, 2, R]), ("d_rec4", [4, 2 * 512]),
                          ("d_st1", [P, 4]), ("d_stg1", [P, 4])]:
            taps[tn] = nc.dram_tensor(tn, shape, F32, kind="ExternalOutput").ap()

    # internal DRAM bounce buffers for collectives
    def dbuf(name, shape):
        return nc.dram_tensor(name, shape, F32).ap()

    cc1_in = dbuf("cc1_in", [P, 4])
    cc1_out = dbuf("cc1_out", [P, 4])
    cc2_in = nc.dram_tensor("cc2_in", [P, 2 * R], F32R).ap()
    cc2_out = nc.dram_tensor("cc2_out", [2, P, 2 * R], F32R).ap()
    cc3_in = dbuf("cc3_in", [P, 4])
    cc3_out = dbuf("cc3_out", [2, P, 4])
    cc4_in = dbuf("cc4_in", [P, 4])
    cc4_out = dbuf("cc4_out", [P, 4])


    with tile.TileContext(nc) as tc:
        with (
            tc.tile_pool(name="persist", bufs=1) as ppool,
            tc.tile_pool(name="work", bufs=2) as wpool,
            tc.tile_pool(name="adjp", bufs=3) as adjpool,
            tc.tile_pool(name="expp", bufs=2) as expool,
            tc.tile_pool(name="psA", bufs=2, space="PSUM") as psA,
            tc.tile_pool(name="psE", bufs=1, space="PSUM") as psE,
            tc.tile_pool(name="psG", bufs=1, space="PSUM") as psG,
        ):
            # ---------- constants ----------
            W_sb = ppool.tile([P, 2, C], F32R, tag="W_sb")
            nc.sync.dma_start(W_sb[:], W.rearrange("(o p) c -> p o c", p=P))
            Wh_sb = ppool.tile([P, 2, C], F32R, tag="Wh_sb")
            nc.sync.dma_start(Wh_sb[:], Whp.rearrange("(o p) c -> p o c", p=P))
            Wk_sb = ppool.tile([P, 2, 3, C], F32R, tag="Wk_sb")
            for k in range(3):
                nc.sync.dma_start(
                    Wk_sb[:, :, k, :],
                    WkT[k].rearrange("(o p) c -> p o c", p=P),
                )
            bs_sb = ppool.tile([P, 2], F32, tag="bs_sb")
            nc.sync.dma_start(bs_sb[:], bs.rearrange("(o p) -> p o", p=P))
            g1_sb = ppool.tile([P, 2], F32, tag="g1_sb")
            nc.sync.dma_start(g1_sb[:], g1.rearrange("(o p) -> p o", p=P))
            b1_sb = ppool.tile([P, 2], F32, tag="b1_sb")
            nc.sync.dma_start(b1_sb[:], b1.rearrange("(o p) -> p o", p=P))
            g2_sb = ppool.tile([P, 2], F32, tag="g2_sb")
            nc.sync.dma_start(g2_sb[:], g2.rearrange("(o p) -> p o", p=P))
            b2_sb = ppool.tile([P, 2], F32, tag="b2_sb")
            nc.sync.dma_start(b2_sb[:], b2.rearrange("(o p) -> p o", p=P))
            Ebc_sb = ppool.tile([4, C], F32R, tag="Ebc_sb")
            nc.sync.dma_start(Ebc_sb[:], Ebc[:, :])
            hm_sb = ppool.tile([P, 2, 2, 2], F32, tag="hm_sb")
            nc.sync.dma_start(hm_sb[:], _bc_ap(hmask[:, :, :]))
            cm40 = ppool.tile([P, 1], F32, tag="cm40")
            nc.vector.memset(cm40[:], -EXP_SHIFT)

            # ---------- phase A: support = x @ W  (support[m, j], m on parts)
            support = ppool.tile([P, 16, C], F32R, tag="big16", name="support")
            for t in range(16):
                ps = psA.tile([P, C], F32, tag="ps", name="ps_sup")
                for ko in range(2):
                    xt = wpool.tile([P, P], F32R, tag="xt")
                    nc.sync.dma_start(
                        xt[:], xT[ko * P : (ko + 1) * P, t * P : (t + 1) * P]
                    )
                    nc.tensor.matmul(
                        ps[:], xt[:], W_sb[:, ko, :],
                        start=(ko == 0), stop=(ko == 1),
                    )
                nc.vector.tensor_copy(out=support[:, t, :], in_=ps[:])

            # ---------- phase B: hT = relu(support^T @ adjT + b)  [j, n_own]
            hT = ppool.tile([P, 2, R], F32, tag="hT_share", name="hT")
            for w in range(2):
                ps_h = [
                    psA.tile([P, 512], F32, tag="ps", name=f"ps_h{o}")
                    for o in range(2)
                ]
                for t in range(16):
                    at = adjpool.tile([P, 512], F32R, tag="at")
                    nc.sync.dma_start(
                        at[:],
                        adjTc[t * P : (t + 1) * P, w * 512 : (w + 1) * 512],
                    )
                    for o in range(2):
                        nc.tensor.matmul(
                            ps_h[o][:],
                            support[:, t, o * P : (o + 1) * P],
                            at[:],
                            start=(t == 0), stop=(t == 15),
                        )
                for o in range(2):
                    nc.scalar.activation(
                        out=hT[:, o, w * 512 : (w + 1) * 512],
                        in_=ps_h[o][:],
                        func=AF.Relu,
                        bias=bs_sb[:, o : o + 1],
                    )

            # ---------- phase C: BN1 stats + allreduce + apply
            st1 = ppool.tile([P, 4], F32, tag="st1")
            sq_scr = wpool.tile([P, R], F32, tag="sq_scr")
            for o in range(2):
                nc.vector.reduce_sum(st1[:, o : o + 1], hT[:, o, :], axis=AX.X)
                nc.scalar.activation(
                    out=sq_scr[:],
                    in_=hT[:, o, :],
                    func=AF.Square,
                    accum_out=st1[:, 2 + o : 3 + o],
                )
            nc.sync.dma_start(cc1_in[:, :], st1[:])
            nc.gpsimd.collective_compute(
                "AllReduce", ALU.add, replica_groups=ALL8,
                ins=[cc1_in.opt()], outs=[cc1_out.opt()],
            )
            stg1 = ppool.tile([P, 4], F32, tag="stg1")
            nc.sync.dma_start(stg1[:], cc1_out[:, :])

            def bn_affine(stg, gam, bet, tagp, fold=1.0):
                """A, C with y = relu(x*A + C) == relu(fold*bn(x))."""
                mean = ppool.tile([P, 2], F32, tag=f"{tagp}_mean")
                nc.vector.tensor_scalar_mul(mean[:], stg[:, 0:2], 1.0 / CNT)
                ex2 = ppool.tile([P, 2], F32, tag=f"{tagp}_ex2")
                nc.vector.tensor_scalar_mul(ex2[:], stg[:, 2:4], 1.0 / CNT)
                var = ppool.tile([P, 2], F32, tag=f"{tagp}_var")
                nc.vector.tensor_tensor(var[:], mean[:], mean[:], ALU.mult)
                nc.vector.tensor_tensor(var[:], ex2[:], var[:], ALU.subtract)
                rstd = ppool.tile([P, 2], F32, tag=f"{tagp}_rstd")
                nc.vector.tensor_scalar_add(var[:], var[:], EPS)
                nc.scalar.activation(rstd[:], var[:], AF.Ln)
                nc.scalar.activation(rstd[:], rstd[:], AF.Exp, scale=-0.5)
                A = ppool.tile([P, 2], F32, tag=f"{tagp}_A")
                nc.vector.tensor_tensor(A[:], gam[:], rstd[:], ALU.mult)
                Cc = ppool.tile([P, 2], F32, tag=f"{tagp}_C")
                nc.vector.tensor_tensor(Cc[:], mean[:], A[:], ALU.mult)
                nc.vector.tensor_tensor(Cc[:], bet[:], Cc[:], ALU.subtract)
                if fold != 1.0:
                    nc.vector.tensor_scalar_mul(A[:], A[:], fold)
                    nc.vector.tensor_scalar_mul(Cc[:], Cc[:], fold)
                return A, Cc

            A1, C1 = bn_affine(stg1, g1_sb, b1_sb, "bn1")
            hbnT = ppool.tile([P, 2, R], F32, tag="hbnT")
            for o in range(2):
                nc.scalar.activation(
                    out=hbnT[:, o, :], in_=hT[:, o, :], func=AF.Relu,
                    scale=A1[:, o : o + 1], bias=C1[:, o : o + 1],
                )

            # f32r copy of hbnT for matmul + gather use
            hbnT_r = ppool.tile([P, 2, R], F32R, tag="hbnT_r")
            nc.vector.tensor_copy(out=hbnT_r[:], in_=hbnT[:])

            # ---------- phase D: pair AllGather of hbnT
            nc.sync.dma_start(cc2_in.rearrange("p (o r) -> p o r", o=2), hbnT_r[:])
            nc.gpsimd.collective_compute(
                "AllGather", ALU.bypass, replica_groups=PAIRS,
                ins=[cc2_in.opt()], outs=[cc2_out.opt()],
            )
            hbnF = ppool.tile([P, 2, 2, R], F32R, tag="hbnF")  # [p, o, src, n]
            for src in range(2):
                nc.sync.dma_start(
                    hbnF[:, :, src, :],
                    cc2_out[src].rearrange("p (o r) -> p o r", o=2),
                )

            # ---------- phase E: hp projections
            # hpT_own: [d-major 128 (= head pair), nq local]
            hpTo = ppool.tile([P, 2, R], F32R, tag="hpTo")
            for hh in range(2):
                for w in range(2):
                    ps = psA.tile([P, 512], F32, tag="ps", name="ps_hpo")
                    for o in range(2):
                        nc.tensor.matmul(
                            ps[:],
                            Wh_sb[:, o, hh * P : (hh + 1) * P],
                            hbnT_r[:, o, w * 512 : (w + 1) * 512],
                            start=(o == 0), stop=(o == 1),
                        )
                    nc.vector.tensor_copy(
                        out=hpTo[:, hh, w * 512 : (w + 1) * 512], in_=ps[:]
                    )
            # hpT_full: [d-major, nk gathered 2048]
            hpTf = ppool.tile([P, 2, 2 * R], F32R, tag="big16", name="hpTf")
            for hh in range(2):
                for src in range(2):
                    for w in range(2):
                        ps = psA.tile([P, 512], F32, tag="ps", name="ps_hpf")
                        for o in range(2):
                            nc.tensor.matmul(
                                ps[:],
                                Wh_sb[:, o, hh * P : (hh + 1) * P],
                                hbnF[:, o, src, w * 512 : (w + 1) * 512],
                                start=(o == 0), stop=(o == 1),
                            )
                        nc.vector.tensor_copy(
                            out=hpTf[
                                :, hh, src * R + w * 512 : src * R + (w + 1) * 512
                            ],
                            in_=ps[:],
                        )
            # hp node-major with ones columns: [nk, hh, 130]
            # cols 0:64 even head, 64 ones, 65:129 odd head, 129 ones
            # even lhsT = cols 0:65, odd lhsT = cols 65:130 -> both give
            # agg at psum partitions 0..63 and denominator at partition 64
            hpA = ppool.tile([P, 16, 2, 130], F32R, tag="hpA")
            ones1 = ppool.tile([P, 1], F32, tag="ones1")
            nc.vector.memset(ones1[:], 1.0)
            ones_src = bass.AP(
                tensor=ones1.tensor, offset=ones1.offset,
                ap=[ones1.ap[0], [0, 16], [0, 2]],
            )
            for col in (64, 129):
                onesv = bass.AP(
                    tensor=hpA.tensor, offset=hpA.offset + col,
                    ap=[hpA.ap[0], [260, 16], [130, 2]],
                )  # [p, t, hh] at fixed col
                nc.vector.tensor_copy(out=onesv, in_=ones_src)
            for t in range(16):
                src, wi = t // 8, t % 8
                ps = psA.tile([P, C], F32, tag="ps", name="ps_hpa")
                for o in range(2):
                    nc.tensor.matmul(
                        ps[:],
                        hbnF[:, o, src, wi * P : (wi + 1) * P],
                        Wh_sb[:, o, :],
                        start=(o == 0), stop=(o == 1),
                    )
                psv = ps.rearrange("p (a e d) -> p a e d", a=2, e=2)
                nc.vector.tensor_copy(out=hpA[:, t, :, 0:64], in_=psv[:, :, 0, :])
                nc.vector.tensor_copy(out=hpA[:, t, :, 65:129], in_=psv[:, :, 1, :])

            # ---------- phase F: attention per head ----------
            g_ext = ppool.tile([P, 2, R + 2], F32, tag="g_ext")
            den4 = ppool.tile([4, 2 * 512], F32, tag="den4")
            aggT = ppool.tile([P, 2, R], F32, tag="aggT")

            for h in range(H):
                hh, eo = h // 2, h % 2
                dlo = eo * 64
                for w in range(2):
                    agg = psG.tile([P, 512], F32, tag="agg")
                    for g in range(4):
                        eg = psE.tile([P, 4, 512], F32, tag="eg")
                        for i in range(4):
                            t = 4 * g + i
                            nc.tensor.matmul(
                                eg[:, i, :],
                                hpTf[dlo : dlo + 64, hh, t * P : (t + 1) * P],
                                hpTo[dlo : dlo + 64, hh, w * 512 : (w + 1) * 512],
                                start=True, stop=True,
                            )
                        el = expool.tile([P, 4, 512], F32R, tag="el")
                        if sim_safe:
                            # CoreSim has no Lrelu: lrelu(x)=relu(x)-a*relu(-x)
                            scr = expool.tile([P, 4, 512], F32, tag="lrscr")
                            nc.scalar.activation(
                                out=el[:], in_=eg[:], func=AF.Relu
                            )
                            nc.scalar.activation(
                                out=scr[:], in_=eg[:], func=AF.Relu, scale=-1.0
                            )
                            nc.vector.tensor_scalar_mul(scr[:], scr[:], -SLOPE)
                            nc.vector.tensor_tensor(
                                el[:], el[:], scr[:], ALU.add
                            )
                        else:
                            # Prelu honors alpha; Lrelu is hardwired to 0.01
                            nc.scalar.activation(
                                out=el[:], in_=eg[:], func=AF.Prelu, alpha=SLOPE
                            )
                        nc.scalar.activation(
                            out=el[:], in_=el[:], func=AF.Exp, bias=cm40[:]
                        )
                        for i in range(4):
                            t = 4 * g + i
                            lhs = hpA[:, t, hh, eo * 65 : eo * 65 + 65]
                            nc.tensor.matmul(
                                agg[0:65, :], lhs, el[:, i, :],
                                start=(t == 0), stop=(t == 15),
                            )
                    # denominator row (psum partition 64) -> den4[h]
                    dstage = wpool.tile([P, 512], F32, tag="stage", name="dstage")
                    nc.vector.tensor_copy(
                        out=dstage[64:65, :], in_=agg[64:65, :]
                    )
                    nc.sync.dma_start(
                        den4[h : h + 1, w * 512 : (w + 1) * 512],
                        dstage[64:65, :],
                    )
                    # agg rows (psum partitions 0..63) -> aggT[dlo:dlo+64]
                    if eo == 0:
                        nc.vector.tensor_copy(
                            out=aggT[0:64, hh, w * 512 : (w + 1) * 512],
                            in_=agg[0:64, :],
                        )
                    else:
                        astage = wpool.tile([P, 512], F32, tag="stage", name="astage")
                        nc.vector.tensor_copy(
                            out=astage[0:64, :], in_=agg[0:64, :]
                        )
                        nc.sync.dma_start(
                            aggT[64:128, hh, w * 512 : (w + 1) * 512],
                            astage[0:64, :],
                        )

            # recip4 = alpha_gat / den  (DVE approx reciprocal, ~2 ULP)
            rec4f = ppool.tile([4, 2 * 512], F32, tag="rec4f")
            rscr = ppool.tile([4, 2 * 512], F32, tag="rscr")
            nc.vector.reciprocal_approx_accurate(
                out=rec4f[:], in_=den4[:], scratch=rscr[:]
            )
            nc.vector.tensor_scalar_mul(rec4f[:], rec4f[:], float(alpha_gat))
            rec4 = ppool.tile([4, 2 * 512], F32R, tag="rec4")
            nc.vector.tensor_copy(out=rec4[:], in_=rec4f[:])
            # g = aggT * recip_bc + (1-alpha)*hbnT   -> g_ext[:, :, 1:R+1]
            for o in range(2):
                for w in range(2):
                    bc = psG.tile([P, 512], F32, tag="bc")
                    nc.tensor.matmul(
                        bc[:],
                        Ebc_sb[:, o * P : (o + 1) * P],
                        rec4[:, w * 512 : (w + 1) * 512],
                        start=True, stop=True,
                    )
                    gsl = g_ext[:, o, 1 + w * 512 : 1 + (w + 1) * 512]
                    nc.vector.tensor_tensor(
                        gsl, aggT[:, o, w * 512 : (w + 1) * 512], bc[:], ALU.mult
                    )
                    from concourse import dve_ops
                    nc.vector._custom_dve(
                        dve_ops.AFFINE_THEN_ADD,
                        out=gsl,
                        in0=hbnT[:, o, w * 512 : (w + 1) * 512],
                        in1=gsl,
                        s0=float(1.0 - alpha_gat),
                        s1=0.0,
                    )

            if debug_taps:
                nc.sync.dma_start(taps["d_hT"][:, :, :], hT[:])
                nc.sync.dma_start(taps["d_hbnT"][:, :, :], hbnT[:])
                hpTo_f = wpool.tile([P, 2, R], F32, tag="hpTo_f")
                nc.vector.tensor_copy(out=hpTo_f[:], in_=hpTo[:])
                nc.sync.dma_start(taps["d_hpTo"][:, :, :], hpTo_f[:])
                nc.sync.dma_start(taps["d_aggT"][:, :, :], aggT[:])
                nc.sync.dma_start(taps["d_den4"][:, :], den4[:])
                rec4_f = wpool.tile([4, 2 * 512], F32, tag="rec4_f")
                nc.vector.tensor_copy(out=rec4_f[:], in_=rec4[:])
                nc.sync.dma_start(taps["d_rec4"][:, :], rec4_f[:])
                nc.sync.dma_start(taps["d_st1"][:, :], st1[:])
                nc.sync.dma_start(taps["d_stg1"][:, :], stg1[:])

            # ---------- phase G: halo exchange of g boundary columns
            # cc3_in cols: [first o0, first o1, last o0, last o1]
            nc.sync.dma_start(
                cc3_in.rearrange("p (f o) -> p f o", f=2)[:, 0, :], g_ext[:, :, 1]
            )
            nc.sync.dma_start(
                cc3_in.rearrange("p (f o) -> p f o", f=2)[:, 1, :], g_ext[:, :, R]
            )
            nc.gpsimd.collective_compute(
                "AllGather", ALU.bypass, replica_groups=PAIRS,
                ins=[cc3_in.opt()], outs=[cc3_out.opt()],
            )
            hal = ppool.tile([P, 2, 2, 2], F32, tag="hal")  # [p, src, f/l, o]
            for src in range(2):
                nc.sync.dma_start(
                    hal[:, src, :, :],
                    cc3_out[src].rearrange("p (f o) -> p f o", f=2),
                )
            # halo[L/R][p, o] = sum_{src, fl} hal[p, src, fl, o] * hmask[LR, src, fl]
            halv = bass.AP(
                tensor=hal.tensor, offset=hal.offset,
                ap=[hal.ap[0], [1, 2], [4, 2], [2, 2]],
            )  # [p, o, src, fl]
            for lr, col in ((0, 0), (1, R + 1)):
                mv = bass.AP(
                    tensor=hm_sb.tensor, offset=hm_sb.offset + lr * 4,
                    ap=[hm_sb.ap[0], [0, 2], [2, 2], [1, 2]],
                )  # [p, o(bc), src, fl]
                tmp = wpool.tile([P, 2, 2, 2], F32, tag="haltmp")
                nc.vector.tensor_tensor(tmp[:], halv, mv, ALU.mult)
                nc.vector.reduce_sum(g_ext[:, :, col], tmp[:], axis=AX.XY)

            # ---------- phase H: TCN conv ----------
            gr_ext = ppool.tile([P, 2, R + 2], F32R, tag="gr_ext")
            nc.vector.tensor_copy(out=gr_ext[:], in_=g_ext[:])
            tconv = ppool.tile([P, 2, R], F32, tag="hT_share", name="tconv")
            for oo in range(2):
                for w in range(2):
                    ps = psA.tile([P, 512], F32, tag="ps", name="ps_cv")
                    first = True
                    for oi in range(2):
                        for k in range(3):
                            nc.tensor.matmul(
                                ps[:],
                                Wk_sb[:, oi, k, oo * P : (oo + 1) * P],
                                gr_ext[:, oi, w * 512 + k : w * 512 + k + 512],
                                start=first, stop=(oi == 1 and k == 2),
                            )
                            first = False
                    nc.vector.tensor_copy(
                        out=tconv[:, oo, w * 512 : (w + 1) * 512], in_=ps[:]
                    )

            if debug_taps:
                nc.sync.dma_start(taps["d_g"][:, :, :], g_ext[:])
                nc.sync.dma_start(taps["d_tconv"][:, :, :], tconv[:])

            # ---------- phase I: BN2 + residual + output ----------
            st2 = ppool.tile([P, 4], F32, tag="st2")
            sq2 = wpool.tile([P, R], F32, tag="sq_scr", name="sq2")
            for o in range(2):
                nc.vector.reduce_sum(st2[:, o : o + 1], tconv[:, o, :], axis=AX.X)
                nc.scalar.activation(
                    out=sq2[:], in_=tconv[:, o, :], func=AF.Square,
                    accum_out=st2[:, 2 + o : 3 + o],
                )
            nc.sync.dma_start(cc4_in[:, :], st2[:])
            nc.gpsimd.collective_compute(
                "AllReduce", ALU.add, replica_groups=ALL8,
                ins=[cc4_in.opt()], outs=[cc4_out.opt()],
            )
            stg2 = ppool.tile([P, 4], F32, tag="stg2")
            nc.sync.dma_start(stg2[:], cc4_out[:, :])
            fold = alpha_tcn if alpha_tcn > 0 else 1.0
            A2, C2 = bn_affine(stg2, g2_sb, b2_sb, "bn2", fold=fold)

            final = ppool.tile([P, 2, R], F32, tag="final")
            from concourse import dve_ops
            for o in range(2):
                nc.scalar.activation(
                    out=final[:, o, :], in_=tconv[:, o, :], func=AF.Relu,
                    scale=A2[:, o : o + 1], bias=C2[:, o : o + 1],
                )
                if fold != alpha_tcn:  # alpha_tcn <= 0: scale separately
                    nc.vector.tensor_scalar_mul(
                        final[:, o, :], final[:, o, :], float(alpha_tcn)
                    )
                nc.vector._custom_dve(
                    dve_ops.AFFINE_THEN_ADD,
                    out=final[:, o, :],
                    in0=g_ext[:, o, 1 : R + 1],
                    in1=final[:, o, :],
                    s0=float(1.0 - alpha_tcn),
                    s1=0.0,
                )
                nc.sync.dma_start(out[:, o, :], final[:, o, :])

    nc.compile()
    return nc


def _f32r(a):
    """Round f32 to the fp32r grid (11-bit mantissa) so DMA'd data matches
    what the PE consumes; lets F32R DRAM tensors skip casting DMAs."""
    a = np.ascontiguousarray(a, np.float32)
    b = a.view(np.uint32).astype(np.uint64)
    b = ((b + 0x800) & 0xFFFFF000).astype(np.uint32)
    return b.view(np.float32)


def _prep_inputs(x, adj, W_sage, b_sage, bn1_gamma, bn1_beta, Wh,
                 conv_w, bn2_gamma, bn2_beta):
    """Build the 8 per-core input maps (host-side numpy)."""
    x = np.asarray(x, np.float32)
    adj = np.asarray(adj, np.float32)
    Whp = np.ascontiguousarray(
        np.asarray(Wh, np.float32).transpose(1, 0, 2).reshape(C, H * DH)
    )
    WkT = np.ascontiguousarray(np.asarray(conv_w, np.float32).transpose(2, 1, 0))
    Ebc = np.zeros((4, C), np.float32)
    for c in range(C):
        Ebc[(c % P) // 64 + 2 * (c // P), c] = 1.0

    shared = dict(
        W=_f32r(np.asarray(W_sage, np.float32)),
        bs=np.asarray(b_sage, np.float32),
        g1=np.asarray(bn1_gamma, np.float32),
        b1=np.asarray(bn1_beta, np.float32),
        Whp=_f32r(Whp), WkT=_f32r(WkT),
        g2=np.asarray(bn2_gamma, np.float32),
        b2=np.asarray(bn2_beta, np.float32),
        Ebc=_f32r(Ebc),
    )
    in_maps = []
    for core in range(NC):
        b, s = core // 2, core % 2
        hmask = np.zeros((2, 2, 2), np.float32)  # [L/R, src, first/last]
        if s == 0:
            hmask[1, 1, 0] = 1.0  # right halo = partner(rank1) first col
        else:
            hmask[0, 0, 1] = 1.0  # left halo = partner(rank0) last col
        m = dict(
            xT=_f32r(x[b].T),
            adjTc=_f32r(adj[s * R : (s + 1) * R, :].T),
            hmask=hmask,
            **shared,
        )
        in_maps.append(m)
    return in_maps


def _assemble(results):
    out = np.empty((B, N, C), np.float32)
    for core in range(NC):
        b, s = core // 2, core % 2
        r = results[core]["out"]  # [P, 2, R]
        out[b, s * R : (s + 1) * R, :] = r.transpose(2, 1, 0).reshape(R, C)
    return out


_CACHE = {}


def kernel(x, adj, W_sage, b_sage, bn1_gamma, bn1_beta, Wh, alpha_gat,
           conv_w, conv_b, bn2_gamma, bn2_beta, alpha_tcn, **_unused):
    ag, at = float(alpha_gat), float(alpha_tcn)
    key = (ag, at)
    if key not in _CACHE:
        _CACHE[key] = build_program(ag, at)
    nc = _CACHE[key]
    in_maps = _prep_inputs(x, adj, W_sage, b_sage, bn1_gamma, bn1_beta, Wh,
                           conv_w, bn2_gamma, bn2_beta)
    res = run_bass_kernel_spmd(nc, in_maps, core_ids=list(range(NC)))
    return _assemble(res.results)


if __name__ == "__main__":
    import sys
    sys.path.insert(0, "/root/problem")
    import reference
    inputs = {k: np.asarray(v) for k, v in reference.setup_inputs().items()}
    expected = np.asarray(reference.reference(**inputs))
    actual = kernel(**inputs)
    err = np.abs(actual - expected)
    rel = np.linalg.norm(actual - expected) / np.linalg.norm(expected)
    print("max abs err:", err.max(), "rel:", rel)


# revision 29
# speedup vs baseline: 257.4655x; 257.4655x over previous
"""Trainium2 Bass kernel for nn_GCNWithMultiHeadGATAndTCN_42356967473538.

Sharding: 8 cores = (batch b in 0..3) x (node-half s in 0..1).
Each core computes its 1024 node rows of its batch through the whole
pipeline, channels-major ([channel partitions, node free]) so BatchNorm
scales are per-partition and the TCN conv contracts on partitions.

Cross-core communication (training-mode BatchNorm couples all batches):
  C1: AllReduce [128,4]   bn1 sums           (all 8 cores)
  C2: AllGather [128,2048] h_bn^T            (pairs: other node half)
  C3: AllGather [128,4]   g boundary columns (pairs: conv halo)
  C4: AllReduce [128,4]   bn2 sums           (all 8 cores)
"""

import numpy as np

import concourse.bass as bass
import concourse.mybir as mybir
import concourse.tile as tile
from concourse import bacc, dve_ops
from concourse.bass_utils import run_bass_kernel_spmd
from concourse.dve_spec import Spec, Src0, C0, maxx, lower, _has_src1
from concourse.dve_uop import DveOpSpec
from concourse.dve_table_gen import dve_ver_for


def _register_lrelu_op():
    """Custom single-pass DVE leaky-relu: out = max(in0, in0*s0)."""
    if "LRELU_ANT" in dve_ops._SUB_OPCODE_FOR_NAME:
        return dve_ops.CUSTOM_DVE_SPECS and next(
            op for op in dve_ops.OPS if op.name == "LRELU_ANT"
        )
    spec = Spec(
        body=maxx(Src0, Src0 * C0),
        reference=lambda in0, in1, s0, s1, imm2: np.maximum(
            np.nan_to_num(in0, nan=0.0, posinf=np.inf, neginf=-np.inf),
            in0 * s0,
        ).astype(np.float32),
    )
    row = dve_ops._CUSTOM_DVE_ROW_BASE + len(dve_ops.OPS)
    assert row < 0x20
    shas = {}
    for ver in ("v3", "v4"):
        try:
            tmp = DveOpSpec(name="LRELU_ANT", opcode=row, uops=lower(spec, ver=ver),
                            rd1_en=_has_src1(spec))
            shas[ver] = tmp.sha(ver)
        except Exception:
            pass
    op = dve_ops.DveOp("LRELU_ANT", spec, False, shas)
    dve_ops.OPS.append(op)
    dve_ops.CUSTOM_DVE_SPECS["LRELU_ANT"] = spec
    dve_ops._SUB_OPCODE_FOR_NAME["LRELU_ANT"] = row
    return op


LRELU_ANT = _register_lrelu_op()

F32 = mybir.dt.float32
F32R = mybir.dt.float32r
AF = mybir.ActivationFunctionType
ALU = mybir.AluOpType
AX = mybir.AxisListType

B, N, FEAT, C, H, DH = 4, 2048, 256, 256, 4, 64
P = 128
R = N // 2            # own rows per core (1024)
NC = 8                # cores
EPS = 1e-5
SLOPE = 0.2
EXP_SHIFT = 64.0  # softmax-invariant constant shift: keeps exp in f32 range
CNT = float(B * N)    # batchnorm sample count (8192)

PAIRS = [[0, 1], [2, 3], [4, 5], [6, 7]]
ALL8 = [list(range(NC))]


def _bc_ap(ap, parts=P):
    """Broadcast a DRAM AP across `parts` partitions (stride-0 partition dim)."""
    return bass.AP(tensor=ap.tensor, offset=ap.offset, ap=[[0, parts], *ap.ap])


def build_program(alpha_gat: float, alpha_tcn: float, sim_safe: bool = False,
                  debug_taps: bool = False):
    nc = bacc.Bacc(
        "TRN2", target_bir_lowering=False, debug=False, num_devices=NC
    )

    def din(name, shape, dt=F32):
        return nc.dram_tensor(name, shape, dt, kind="ExternalInput").ap()

    xT = din("xT", [FEAT, N], F32R)      # x[b].T
    adjTc = din("adjTc", [N, R], F32R)   # adj[s*R:(s+1)*R, :].T  (own columns)
    W = din("W", [FEAT, C], F32R)        # W_sage
    bs = din("bs", [C])
    g1 = din("g1", [C])
    b1 = din("b1", [C])
    Whp = din("Whp", [C, H * DH], F32R)        # Wh packed [j, h*64+d]
    WkT = din("WkT", [3, C, C], F32R)          # conv_w[:, :, k].T -> [k, cin, cout]
    g2 = din("g2", [C])
    b2 = din("b2", [C])
    Ebc = din("Ebc", [4, C], F32R)             # head->channel one-hot (recip bcast)
    hmask = din("hmask", [2, 2, 2])      # halo select [L/R, src, first/last]

    out = nc.dram_tensor("out", [P, 2, R], F32, kind="ExternalOutput").ap()
    taps = {}
    if debug_taps:
        for tn, shape in [("d_hT", [P, 2, R]), ("d_hbnT", [P, 2, R]),
                          ("d_hpTo", [P, 2, R]), ("d_aggT", [P, 2, R]),
                          ("d_den4", [4, 2 * 512]), ("d_g", [P, 2, R + 2]),
                          ("d_tconv", [P, 2, R]), ("d_rec4", [4, 2 * 512]),
                          ("d_st1", [P, 4]), ("d_stg1", [P, 4])]:
            taps[tn] = nc.dram_tensor(tn, shape, F32, kind="ExternalOutput").ap()

    # internal DRAM bounce buffers for collectives
    def dbuf(name, shape):
        return nc.dram_tensor(name, shape, F32).ap()

    cc1_in = dbuf("cc1_in", [P, 4])
    cc1_out = dbuf("cc1_out", [P, 4])
    cc2_in = nc.dram_tensor("cc2_in", [P, 2 * R], F32R).ap()
    cc2_out = nc.dram_tensor("cc2_out", [2, P, 2 * R], F32R).ap()
    cc3_in = dbuf("cc3_in", [P, 4])
    cc3_out = dbuf("cc3_out", [2, P, 4])
    cc4_in = dbuf("cc4_in", [P, 4])
    cc4_out = dbuf("cc4_out", [P, 4])


    with tile.TileContext(nc) as tc:
        with (
            tc.tile_pool(name="persist", bufs=1) as ppool,
            tc.tile_pool(name="work", bufs=2) as wpool,
            tc.tile_pool(name="adjp", bufs=3) as adjpool,
            tc.tile_pool(name="expp", bufs=3) as expool,
            tc.tile_pool(name="psum", bufs=1, space="PSUM") as psum,
        ):
            # ---------- constants ----------
            W_sb = ppool.tile([P, 2, C], F32R, tag="W_sb")
            nc.sync.dma_start(W_sb[:], W.rearrange("(o p) c -> p o c", p=P))
            Wh_sb = ppool.tile([P, 2, C], F32R, tag="Wh_sb")
            nc.sync.dma_start(Wh_sb[:], Whp.rearrange("(o p) c -> p o c", p=P))
            Wk_sb = ppool.tile([P, 2, 3, C], F32R, tag="Wk_sb")
            for k in range(3):
                nc.sync.dma_start(
                    Wk_sb[:, :, k, :],
                    WkT[k].rearrange("(o p) c -> p o c", p=P),
                )
            bs_sb = ppool.tile([P, 2], F32, tag="bs_sb")
            nc.sync.dma_start(bs_sb[:], bs.rearrange("(o p) -> p o", p=P))
            g1_sb = ppool.tile([P, 2], F32, tag="g1_sb")
            nc.sync.dma_start(g1_sb[:], g1.rearrange("(o p) -> p o", p=P))
            b1_sb = ppool.tile([P, 2], F32, tag="b1_sb")
            nc.sync.dma_start(b1_sb[:], b1.rearrange("(o p) -> p o", p=P))
            g2_sb = ppool.tile([P, 2], F32, tag="g2_sb")
            nc.sync.dma_start(g2_sb[:], g2.rearrange("(o p) -> p o", p=P))
            b2_sb = ppool.tile([P, 2], F32, tag="b2_sb")
            nc.sync.dma_start(b2_sb[:], b2.rearrange("(o p) -> p o", p=P))
            Ebc_sb = ppool.tile([4, C], F32R, tag="Ebc_sb")
            nc.sync.dma_start(Ebc_sb[:], Ebc[:, :])
            hm_sb = ppool.tile([P, 2, 2, 2], F32, tag="hm_sb")
            nc.sync.dma_start(hm_sb[:], _bc_ap(hmask[:, :, :]))
            cm40 = ppool.tile([P, 1], F32, tag="cm40")
            nc.vector.memset(cm40[:], -EXP_SHIFT)

            # ---------- phase A: support = x @ W  (support[m, j], m on parts)
            support = ppool.tile([P, 16, C], F32R, tag="big16", name="support")
            xTv = xT.rearrange("(ko p) m -> p ko m", p=P)
            for t in range(16):
                ps = psum.tile([P, C], F32, tag=f"q{t % 2}", name="ps_sup")
                xt = wpool.tile([P, 2, P], F32R, tag="xt")
                nc.sync.dma_start(xt[:], xTv[:, :, t * P : (t + 1) * P])
                for ko in range(2):
                    nc.tensor.matmul(
                        ps[:], xt[:, ko, :], W_sb[:, ko, :],
                        start=(ko == 0), stop=(ko == 1),
                    )
                nc.vector.tensor_copy(out=support[:, t, :], in_=ps[:])

            # ---------- phase B: hT = relu(support^T @ adjT + b)  [j, n_own]
            hT = ppool.tile([P, 2, R], F32, tag="hT_share", name="hT")
            ps_h = [
                [
                    psum.tile([P, 512], F32, tag=f"q{o * 2 + w}", name=f"ps_h{o}{w}")
                    for w in range(2)
                ]
                for o in range(2)
            ]
            for t in range(16):
                at = adjpool.tile([P, R], F32R, tag="at")
                nc.sync.dma_start(at[:], adjTc[t * P : (t + 1) * P, :])
                for o in range(2):
                    for w in range(2):
                        nc.tensor.matmul(
                            ps_h[o][w][:],
                            support[:, t, o * P : (o + 1) * P],
                            at[:, w * 512 : (w + 1) * 512],
                            start=(t == 0), stop=(t == 15),
                        )
            for o in range(2):
                for w in range(2):
                    nc.scalar.activation(
                        out=hT[:, o, w * 512 : (w + 1) * 512],
                        in_=ps_h[o][w][:],
                        func=AF.Relu,
                        bias=bs_sb[:, o : o + 1],
                    )

            # ---------- phase C: BN1 stats + allreduce + apply
            st1 = ppool.tile([P, 4], F32, tag="st1")
            sq_scr = wpool.tile([P, R], F32, tag="sq_scr")
            for o in range(2):
                nc.vector.reduce_sum(st1[:, o : o + 1], hT[:, o, :], axis=AX.X)
                nc.scalar.activation(
                    out=sq_scr[:],
                    in_=hT[:, o, :],
                    func=AF.Square,
                    accum_out=st1[:, 2 + o : 3 + o],
                )
            nc.sync.dma_start(cc1_in[:, :], st1[:])
            nc.gpsimd.collective_compute(
                "AllReduce", ALU.add, replica_groups=ALL8,
                ins=[cc1_in.opt()], outs=[cc1_out.opt()],
            )
            stg1 = ppool.tile([P, 4], F32, tag="stg1")
            nc.sync.dma_start(stg1[:], cc1_out[:, :])

            def bn_affine(stg, gam, bet, tagp, fold=1.0):
                """A, C with y = relu(x*A + C) == relu(fold*bn(x))."""
                mean = ppool.tile([P, 2], F32, tag=f"{tagp}_mean")
                nc.vector.tensor_scalar_mul(mean[:], stg[:, 0:2], 1.0 / CNT)
                ex2 = ppool.tile([P, 2], F32, tag=f"{tagp}_ex2")
                nc.vector.tensor_scalar_mul(ex2[:], stg[:, 2:4], 1.0 / CNT)
                var = ppool.tile([P, 2], F32, tag=f"{tagp}_var")
                nc.vector.tensor_tensor(var[:], mean[:], mean[:], ALU.mult)
                nc.vector.tensor_tensor(var[:], ex2[:], var[:], ALU.subtract)
                rstd = ppool.tile([P, 2], F32, tag=f"{tagp}_rstd")
                nc.vector.tensor_scalar_add(var[:], var[:], EPS)
                nc.scalar.activation(rstd[:], var[:], AF.Ln)
                nc.scalar.activation(rstd[:], rstd[:], AF.Exp, scale=-0.5)
                A = ppool.tile([P, 2], F32, tag=f"{tagp}_A")
                nc.vector.tensor_tensor(A[:], gam[:], rstd[:], ALU.mult)
                Cc = ppool.tile([P, 2], F32, tag=f"{tagp}_C")
                nc.vector.tensor_tensor(Cc[:], mean[:], A[:], ALU.mult)
                nc.vector.tensor_tensor(Cc[:], bet[:], Cc[:], ALU.subtract)
                if fold != 1.0:
                    nc.vector.tensor_scalar_mul(A[:], A[:], fold)
                    nc.vector.tensor_scalar_mul(Cc[:], Cc[:], fold)
                return A, Cc

            A1, C1 = bn_affine(stg1, g1_sb, b1_sb, "bn1")
            hbnT = ppool.tile([P, 2, R], F32, tag="hbnT")
            for o in range(2):
                nc.scalar.activation(
                    out=hbnT[:, o, :], in_=hT[:, o, :], func=AF.Relu,
                    scale=A1[:, o : o + 1], bias=C1[:, o : o + 1],
                )

            # f32r copy of hbnT for matmul + gather use
            hbnT_r = ppool.tile([P, 2, R], F32R, tag="hbnT_r")
            nc.vector.tensor_copy(out=hbnT_r[:], in_=hbnT[:])

            # ---------- phase D: pair AllGather of hbnT
            nc.sync.dma_start(cc2_in.rearrange("p (o r) -> p o r", o=2), hbnT_r[:])
            nc.gpsimd.collective_compute(
                "AllGather", ALU.bypass, replica_groups=PAIRS,
                ins=[cc2_in.opt()], outs=[cc2_out.opt()],
            )
            hbnF = ppool.tile([P, 2, 2, R], F32R, tag="hbnF")  # [p, o, src, n]
            for src in range(2):
                nc.sync.dma_start(
                    hbnF[:, :, src, :],
                    cc2_out[src].rearrange("p (o r) -> p o r", o=2),
                )

            # ---------- phase E: hp projections
            # hpT_own: [d-major 128 (= head pair), nq local]
            hpTo = ppool.tile([P, 2, R], F32R, tag="hpTo")
            for hh in range(2):
                for w in range(2):
                    ps = psum.tile([P, 512], F32, tag=f"q{(hh * 2 + w) % 2}", name="ps_hpo")
                    for o in range(2):
                        nc.tensor.matmul(
                            ps[:],
                            Wh_sb[:, o, hh * P : (hh + 1) * P],
                            hbnT_r[:, o, w * 512 : (w + 1) * 512],
                            start=(o == 0), stop=(o == 1),
                        )
                    nc.vector.tensor_copy(
                        out=hpTo[:, hh, w * 512 : (w + 1) * 512], in_=ps[:]
                    )
            # hpT_full: [d-major, nk gathered 2048]
            hpTf = ppool.tile([P, 2, 2 * R], F32R, tag="big16", name="hpTf")
            for hh in range(2):
                for src in range(2):
                    for w in range(2):
                        ps = psum.tile([P, 512], F32, tag=f"q{(src * 2 + w) % 2}", name="ps_hpf")
                        for o in range(2):
                            nc.tensor.matmul(
                                ps[:],
                                Wh_sb[:, o, hh * P : (hh + 1) * P],
                                hbnF[:, o, src, w * 512 : (w + 1) * 512],
                                start=(o == 0), stop=(o == 1),
                            )
                        nc.vector.tensor_copy(
                            out=hpTf[
                                :, hh, src * R + w * 512 : src * R + (w + 1) * 512
                            ],
                            in_=ps[:],
                        )
            # hp node-major with ones columns: [nk, hh, 130]
            # cols 0:64 even head, 64 ones, 65:129 odd head, 129 ones
            # even lhsT = cols 0:65, odd lhsT = cols 65:130 -> both give
            # agg at psum partitions 0..63 and denominator at partition 64
            hpA = ppool.tile([P, 16, 2, 130], F32R, tag="hpA")
            ones1 = ppool.tile([P, 1], F32, tag="ones1")
            nc.vector.memset(ones1[:], 1.0)
            ones_src = bass.AP(
                tensor=ones1.tensor, offset=ones1.offset,
                ap=[ones1.ap[0], [0, 16], [0, 2]],
            )
            for col in (64, 129):
                onesv = bass.AP(
                    tensor=hpA.tensor, offset=hpA.offset + col,
                    ap=[hpA.ap[0], [260, 16], [130, 2]],
                )  # [p, t, hh] at fixed col
                nc.vector.tensor_copy(out=onesv, in_=ones_src)
            for t in range(16):
                src, wi = t // 8, t % 8
                ps = psum.tile([P, C], F32, tag=f"q{t % 2}", name="ps_hpa")
                for o in range(2):
                    nc.tensor.matmul(
                        ps[:],
                        hbnF[:, o, src, wi * P : (wi + 1) * P],
                        Wh_sb[:, o, :],
                        start=(o == 0), stop=(o == 1),
                    )
                psv = ps.rearrange("p (a e d) -> p a e d", a=2, e=2)
                nc.vector.tensor_copy(out=hpA[:, t, :, 0:64], in_=psv[:, :, 0, :])
                nc.vector.tensor_copy(out=hpA[:, t, :, 65:129], in_=psv[:, :, 1, :])

            # ---------- phase F: attention per head ----------
            g_ext = ppool.tile([P, 2, R + 2], F32, tag="g_ext")
            den4 = ppool.tile([4, 2 * 512], F32, tag="den4")
            aggT = ppool.tile([P, 2, R], F32, tag="aggT")

            for hh in range(2):
                for w in range(2):
                    aggE = psum.tile([P, 512], F32, tag="aggE", name="aggE")
                    aggO = psum.tile([P, 512], F32, tag="aggO", name="aggO")
                    for t in range(16):
                        egE = psum.tile(
                            [P, 512], F32, tag=f"q{t % 2}", name="egE"
                        )
                        egO = psum.tile(
                            [P, 512], F32, tag=f"q{2 + t % 2}", name="egO"
                        )
                        # even/odd head e-matmuls adjacent: lhsT partition
                        # bases 0/64 -> tile_position row-packing, concurrent
                        nc.tensor.matmul(
                            egE[:],
                            hpTf[0:64, hh, t * P : (t + 1) * P],
                            hpTo[0:64, hh, w * 512 : (w + 1) * 512],
                            start=True, stop=True,
                        )
                        nc.tensor.matmul(
                            egO[:],
                            hpTf[64:128, hh, t * P : (t + 1) * P],
                            hpTo[64:128, hh, w * 512 : (w + 1) * 512],
                            start=True, stop=True,
                        )
                        # leaky-relu in place on PSUM (keeps f32 precision),
                        # one custom DVE op each
                        nc.vector._custom_dve(
                            LRELU_ANT, out=egE[:], in0=egE[:], s0=SLOPE
                        )
                        nc.vector._custom_dve(
                            LRELU_ANT, out=egO[:], in0=egO[:], s0=SLOPE
                        )
                        elE = expool.tile([P, 512], F32R, tag="elE", name="elE")
                        elO = expool.tile([P, 512], F32R, tag="elO", name="elO")
                        nc.scalar.activation(
                            out=elE[:], in_=egE[:], func=AF.Exp, bias=cm40[:]
                        )
                        nc.scalar.activation(
                            out=elO[:], in_=egO[:], func=AF.Exp, bias=cm40[:]
                        )
                        nc.tensor.matmul(
                            aggE[0:65, :], hpA[:, t, hh, 0:65], elE[:],
                            start=(t == 0), stop=(t == 15),
                        )
                        nc.tensor.matmul(
                            aggO[0:65, :], hpA[:, t, hh, 65:130], elO[:],
                            start=(t == 0), stop=(t == 15),
                        )
                    for eo, agg in ((0, aggE), (1, aggO)):
                        h = 2 * hh + eo
                        dstage = wpool.tile(
                            [P, 512], F32, tag="stage", name="dstage"
                        )
                        nc.vector.tensor_copy(
                            out=dstage[64:65, :], in_=agg[64:65, :]
                        )
                        nc.sync.dma_start(
                            den4[h : h + 1, w * 512 : (w + 1) * 512],
                            dstage[64:65, :],
                        )
                        if eo == 0:
                            nc.scalar.activation(
                                out=aggT[0:64, hh, w * 512 : (w + 1) * 512],
                                in_=agg[0:64, :], func=AF.Copy,
                            )
                        else:
                            astage = wpool.tile(
                                [P, 512], F32, tag="stage", name="astage"
                            )
                            nc.scalar.activation(
                                out=astage[0:64, :], in_=agg[0:64, :],
                                func=AF.Copy,
                            )
                            nc.sync.dma_start(
                                aggT[64:128, hh, w * 512 : (w + 1) * 512],
                                astage[0:64, :],
                            )
            # recip4 = alpha_gat / den  (DVE approx reciprocal, ~2 ULP)
            rec4f = ppool.tile([4, 2 * 512], F32, tag="rec4f")
            rscr = ppool.tile([4, 2 * 512], F32, tag="rscr")
            nc.vector.reciprocal_approx_accurate(
                out=rec4f[:], in_=den4[:], scratch=rscr[:]
            )
            nc.vector.tensor_scalar_mul(rec4f[:], rec4f[:], float(alpha_gat))
            rec4 = ppool.tile([4, 2 * 512], F32R, tag="rec4")
            nc.vector.tensor_copy(out=rec4[:], in_=rec4f[:])
            # g = aggT * recip_bc + (1-alpha)*hbnT   -> g_ext[:, :, 1:R+1]
            for o in range(2):
                for w in range(2):
                    bc = psum.tile([P, 512], F32, tag="q2", name="bc")
                    nc.tensor.matmul(
                        bc[:],
                        Ebc_sb[:, o * P : (o + 1) * P],
                        rec4[:, w * 512 : (w + 1) * 512],
                        start=True, stop=True,
                    )
                    gsl = g_ext[:, o, 1 + w * 512 : 1 + (w + 1) * 512]
                    nc.vector.tensor_tensor(
                        gsl, aggT[:, o, w * 512 : (w + 1) * 512], bc[:], ALU.mult
                    )
                    from concourse import dve_ops
                    nc.vector._custom_dve(
                        dve_ops.AFFINE_THEN_ADD,
                        out=gsl,
                        in0=hbnT[:, o, w * 512 : (w + 1) * 512],
                        in1=gsl,
                        s0=float(1.0 - alpha_gat),
                        s1=0.0,
                    )

            if debug_taps:
                nc.sync.dma_start(taps["d_hT"][:, :, :], hT[:])
                nc.sync.dma_start(taps["d_hbnT"][:, :, :], hbnT[:])
                hpTo_f = wpool.tile([P, 2, R], F32, tag="hpTo_f")
                nc.vector.tensor_copy(out=hpTo_f[:], in_=hpTo[:])
                nc.sync.dma_start(taps["d_hpTo"][:, :, :], hpTo_f[:])
                nc.sync.dma_start(taps["d_aggT"][:, :, :], aggT[:])
                nc.sync.dma_start(taps["d_den4"][:, :], den4[:])
                rec4_f = wpool.tile([4, 2 * 512], F32, tag="rec4_f")
                nc.vector.tensor_copy(out=rec4_f[:], in_=rec4[:])
                nc.sync.dma_start(taps["d_rec4"][:, :], rec4_f[:])
                nc.sync.dma_start(taps["d_st1"][:, :], st1[:])
                nc.sync.dma_start(taps["d_stg1"][:, :], stg1[:])

            # ---------- phase G: halo exchange of g boundary columns
            # cc3_in cols: [first o0, first o1, last o0, last o1]
            nc.sync.dma_start(
                cc3_in.rearrange("p (f o) -> p f o", f=2)[:, 0, :], g_ext[:, :, 1]
            )
            nc.sync.dma_start(
                cc3_in.rearrange("p (f o) -> p f o", f=2)[:, 1, :], g_ext[:, :, R]
            )
            nc.gpsimd.collective_compute(
                "AllGather", ALU.bypass, replica_groups=PAIRS,
                ins=[cc3_in.opt()], outs=[cc3_out.opt()],
            )
            hal = ppool.tile([P, 2, 2, 2], F32, tag="hal")  # [p, src, f/l, o]
            for src in range(2):
                nc.sync.dma_start(
                    hal[:, src, :, :],
                    cc3_out[src].rearrange("p (f o) -> p f o", f=2),
                )
            # halo[L/R][p, o] = sum_{src, fl} hal[p, src, fl, o] * hmask[LR, src, fl]
            halv = bass.AP(
                tensor=hal.tensor, offset=hal.offset,
                ap=[hal.ap[0], [1, 2], [4, 2], [2, 2]],
            )  # [p, o, src, fl]
            for lr, col in ((0, 0), (1, R + 1)):
                mv = bass.AP(
                    tensor=hm_sb.tensor, offset=hm_sb.offset + lr * 4,
                    ap=[hm_sb.ap[0], [0, 2], [2, 2], [1, 2]],
                )  # [p, o(bc), src, fl]
                tmp = wpool.tile([P, 2, 2, 2], F32, tag="haltmp")
                nc.vector.tensor_tensor(tmp[:], halv, mv, ALU.mult)
                nc.vector.reduce_sum(g_ext[:, :, col], tmp[:], axis=AX.XY)

            # ---------- phase H: TCN conv ----------
            gr_ext = ppool.tile([P, 2, R + 2], F32R, tag="gr_ext")
            nc.vector.tensor_copy(out=gr_ext[:], in_=g_ext[:])
            tconv = ppool.tile([P, 2, R], F32, tag="hT_share", name="tconv")
            for oo in range(2):
                for w in range(2):
                    ps = psum.tile([P, 512], F32, tag=f"q{(oo * 2 + w) % 2}", name="ps_cv")
                    first = True
                    for oi in range(2):
                        for k in range(3):
                            nc.tensor.matmul(
                                ps[:],
                                Wk_sb[:, oi, k, oo * P : (oo + 1) * P],
                                gr_ext[:, oi, w * 512 + k : w * 512 + k + 512],
                                start=first, stop=(oi == 1 and k == 2),
                            )
                            first = False
                    nc.vector.tensor_copy(
                        out=tconv[:, oo, w * 512 : (w + 1) * 512], in_=ps[:]
                    )

            if debug_taps:
                nc.sync.dma_start(taps["d_g"][:, :, :], g_ext[:])
                nc.sync.dma_start(taps["d_tconv"][:, :, :], tconv[:])

            # ---------- phase I: BN2 + residual + output ----------
            st2 = ppool.tile([P, 4], F32, tag="st2")
            sq2 = wpool.tile([P, R], F32, tag="sq_scr", name="sq2")
            for o in range(2):
                nc.vector.reduce_sum(st2[:, o : o + 1], tconv[:, o, :], axis=AX.X)
                nc.scalar.activation(
                    out=sq2[:], in_=tconv[:, o, :], func=AF.Square,
                    accum_out=st2[:, 2 + o : 3 + o],
                )
            nc.sync.dma_start(cc4_in[:, :], st2[:])
            nc.gpsimd.collective_compute(
                "AllReduce", ALU.add, replica_groups=ALL8,
                ins=[cc4_in.opt()], outs=[cc4_out.opt()],
            )
            stg2 = ppool.tile([P, 4], F32, tag="stg2")
            nc.sync.dma_start(stg2[:], cc4_out[:, :])
            fold = alpha_tcn if alpha_tcn > 0 else 1.0
            A2, C2 = bn_affine(stg2, g2_sb, b2_sb, "bn2", fold=fold)

            final = ppool.tile([P, 2, R], F32, tag="final")
            from concourse import dve_ops
            for o in range(2):
                nc.scalar.activation(
                    out=final[:, o, :], in_=tconv[:, o, :], func=AF.Relu,
                    scale=A2[:, o : o + 1], bias=C2[:, o : o + 1],
                )
                if fold != alpha_tcn:  # alpha_tcn <= 0: scale separately
                    nc.vector.tensor_scalar_mul(
                        final[:, o, :], final[:, o, :], float(alpha_tcn)
                    )
                nc.vector._custom_dve(
                    dve_ops.AFFINE_THEN_ADD,
                    out=final[:, o, :],
                    in0=g_ext[:, o, 1 : R + 1],
                    in1=final[:, o, :],
                    s0=float(1.0 - alpha_tcn),
                    s1=0.0,
                )
                nc.sync.dma_start(out[:, o, :], final[:, o, :])

    nc.compile()
    return nc


def _f32r(a):
    """Round f32 to the fp32r grid (11-bit mantissa) so DMA'd data matches
    what the PE consumes; lets F32R DRAM tensors skip casting DMAs."""
    a = np.ascontiguousarray(a, np.float32)
    b = a.view(np.uint32).astype(np.uint64)
    b = ((b + 0x800) & 0xFFFFF000).astype(np.uint32)
    return b.view(np.float32)


def _prep_inputs(x, adj, W_sage, b_sage, bn1_gamma, bn1_beta, Wh,
                 conv_w, bn2_gamma, bn2_beta):
    """Build the 8 per-core input maps (host-side numpy)."""
    x = np.asarray(x, np.float32)
    adj = np.asarray(adj, np.float32)
    Whp = np.ascontiguousarray(
        np.asarray(Wh, np.float32).transpose(1, 0, 2).reshape(C, H * DH)
    )
    WkT = np.ascontiguousarray(np.asarray(conv_w, np.float32).transpose(2, 1, 0))
    Ebc = np.zeros((4, C), np.float32)
    for c in range(C):
        Ebc[(c % P) // 64 + 2 * (c // P), c] = 1.0

    shared = dict(
        W=_f32r(np.asarray(W_sage, np.float32)),
        bs=np.asarray(b_sage, np.float32),
        g1=np.asarray(bn1_gamma, np.float32),
        b1=np.asarray(bn1_beta, np.float32),
        Whp=_f32r(Whp), WkT=_f32r(WkT),
        g2=np.asarray(bn2_gamma, np.float32),
        b2=np.asarray(bn2_beta, np.float32),
        Ebc=_f32r(Ebc),
    )
    in_maps = []
    for core in range(NC):
        b, s = core // 2, core % 2
        hmask = np.zeros((2, 2, 2), np.float32)  # [L/R, src, first/last]
        if s == 0:
            hmask[1, 1, 0] = 1.0  # right halo = partner(rank1) first col
        else:
            hmask[0, 0, 1] = 1.0  # left halo = partner(rank0) last col
        m = dict(
            xT=_f32r(x[b].T),
            adjTc=_f32r(adj[s * R : (s + 1) * R, :].T),
            hmask=hmask,
            **shared,
        )
        in_maps.append(m)
    return in_maps


def _assemble(results):
    out = np.empty((B, N, C), np.float32)
    for core in range(NC):
        b, s = core // 2, core % 2
        r = results[core]["out"]  # [P, 2, R]
        out[b, s * R : (s + 1) * R, :] = r.transpose(2, 1, 0).reshape(R, C)
    return out


_CACHE = {}


def kernel(x, adj, W_sage, b_sage, bn1_gamma, bn1_beta, Wh, alpha_gat,
           conv_w, conv_b, bn2_gamma, bn2_beta, alpha_tcn, **_unused):
    ag, at = float(alpha_gat), float(alpha_tcn)
    key = (ag, at)
    if key not in _CACHE:
        _CACHE[key] = build_program(ag, at)
    nc = _CACHE[key]
    in_maps = _prep_inputs(x, adj, W_sage, b_sage, bn1_gamma, bn1_beta, Wh,
                           conv_w, bn2_gamma, bn2_beta)
    res = run_bass_kernel_spmd(nc, in_maps, core_ids=list(range(NC)))
    return _assemble(res.results)


if __name__ == "__main__":
    import sys
    sys.path.insert(0, "/root/problem")
    import reference
    inputs = {k: np.asarray(v) for k, v in reference.setup_inputs().items()}
    expected = np.asarray(reference.reference(**inputs))
    actual = kernel(**inputs)
    err = np.abs(actual - expected)
    rel = np.linalg.norm(actual - expected) / np.linalg.norm(expected)
    print("max abs err:", err.max(), "rel:", rel)
